# revision 21
# baseline (speedup 1.0000x reference)
"""Trainium2 Bass kernel for nn_LinearReg_55508157333593.

Computes: loss = (c_omega * 0.001 / N) * sum over all rows/groups of
L2 norms of 25-element groups of weight [100000, 800] f32.

Since each row is 32 contiguous groups of 25 floats and rows are contiguous,
the whole buffer is just 3.2M consecutive 25-float groups. We shard the flat
array across 8 NeuronCores (10M floats each) and stream each core's slab
through SBUF as [128, 78125] (each partition owns 3125 consecutive groups).

Raw-Bass manual pipeline, per chunk i of the schedule:
  SP:  DMA chunk i into input slot i%B         (per-slot completion sems)
  ACT: square chunk i in place (SBUF->SBUF)
  DVE: per-group (25) reduce into this chunk's slice of gs_all [128, 3125]
Endgame: two batched ACT sqrts over gs_all (bulk + small tail), each with a
fused per-partition row-sum (accum_out -> pr2 column), then PE matmul
ones.T @ pr2 -> PSUM [1, 2], DVE copy to SBUF, single-partition DMA out.
The host sums 8 cores x 2 values in float64 and applies the scaling.
"""

import sys

import numpy as np

if "/opt/trn_rl_repo" not in sys.path:
    sys.path.insert(0, "/opt/trn_rl_repo")

N_CORES = 8
P = 128                      # SBUF partitions
GROUP = 25                   # elements per group
C_OMEGA = 0.001
N_ROWS = 100000
ROW = 800                    # elements per row
F_PER_PART = (N_ROWS * ROW) // (N_CORES * P)   # 78125 floats/partition/core

# chunk schedule (floats per partition; multiples of GROUP, sums to 78125):
# big chunks for streaming, then a descending tail so the serial compute
# chain after the last DMA byte is short.
SCHEDULE = [6250] * 11 + [3125] * 2 + [625] * 4 + [500, 125]
SQRT_SPLIT = 13              # bulk sqrt covers chunks [0, SQRT_SPLIT)

_compiled = None
LAST_RESULTS = None          # BassKernelResults of the most recent run


def build(f_per_part=F_PER_PART, schedule=None, in_bufs=6, sqrt_split=None,
          use_block=False):
    """Build and compile the per-core raw-Bass program."""
    from concourse import bacc, mybir

    if schedule is None:
        schedule = SCHEDULE
    if sqrt_split is None:
        sqrt_split = SQRT_SPLIT if schedule is SCHEDULE else max(1, len(schedule) - 2)
    assert sum(schedule) == f_per_part
    assert all(s % GROUP == 0 for s in schedule)
    n = len(schedule)
    assert 0 < sqrt_split <= n
    offs = [sum(schedule[:i]) for i in range(n)]
    gpcs = [s // GROUP for s in schedule]
    goffs = [sum(gpcs[:i]) for i in range(n)]
    total_g = sum(gpcs)
    split_g = goffs[sqrt_split] if sqrt_split < n else total_g
    n_sqrts = 2 if sqrt_split < n else 1
    max_sz = max(schedule)
    f32 = mybir.dt.float32
    Act = mybir.ActivationFunctionType

    nc = bacc.Bacc("TRN2", target_bir_lowering=False, debug=False,
                   num_devices=N_CORES)
    x = nc.dram_tensor("x", [P, f_per_part], f32, kind="ExternalInput").ap()
    # single-partition output: one small DMA descriptor, fast completion
    out = nc.dram_tensor("out", [1, n_sqrts], f32, kind="ExternalOutput").ap()

    B = in_bufs
    t = [nc.alloc_sbuf_tensor(f"t{b}", [P, max_sz], f32).ap() for b in range(B)]
    gs_all = nc.alloc_sbuf_tensor("gs_all", [P, total_g], f32).ap()
    gn = nc.alloc_sbuf_tensor("gn", [P, total_g], f32).ap()
    pr2 = nc.alloc_sbuf_tensor("pr2", [P, n_sqrts], f32).ap()
    ones = nc.alloc_sbuf_tensor("ones", [P, 1], f32).ap()
    res_sb = nc.alloc_sbuf_tensor("res_sb", [1, n_sqrts], f32).ap()
    ps = nc.alloc_psum_tensor("ps", [1, n_sqrts], f32).ap()

    dma_sems = [nc.alloc_semaphore(f"dma_sem{b}") for b in range(B)]
    out_sem = nc.alloc_semaphore("out_sem")
    sq_sem = nc.alloc_semaphore("sq_sem")       # ACT square i done
    red_sem = nc.alloc_semaphore("red_sem")     # DVE reduce i done
    sqrt_sem = nc.alloc_semaphore("sqrt_sem")   # ACT batched sqrts done
    ones_sem = nc.alloc_semaphore("ones_sem")   # ones vector initialized
    mm_sem = nc.alloc_semaphore("mm_sem")       # PE partition-sum done
    cp_sem = nc.alloc_semaphore("cp_sem")       # PSUM->SBUF copy done

    def emit_gp(gp):
        gp.memset(ones, 1.0).then_inc(ones_sem, 1)

    def emit_sp(sp):
        for i in range(n):
            if i >= B:
                # input slot free once its reduce has consumed it
                sp.wait_ge(red_sem, i - B + 1)
            sp.dma_start(
                t[i % B][:, :schedule[i]], x[:, offs[i]:offs[i] + schedule[i]]
            ).then_inc(dma_sems[i % B], 16)
        sp.wait_ge(cp_sem, 1)
        sp.dma_start(out, res_sb).then_inc(out_sem, 16)
        sp.wait_ge(out_sem, 16)

    def emit_act(act):
        for i in range(n):
            act.wait_ge(dma_sems[i % B], 16 * (i // B + 1))
            act.activation(t[i % B][:, :schedule[i]], t[i % B][:, :schedule[i]],
                           Act.Square).then_inc(sq_sem, 1)
        act.wait_ge(red_sem, sqrt_split)
        act.activation(gn[:, :split_g], gs_all[:, :split_g], Act.Sqrt,
                       accum_out=pr2[:, 0:1]).then_inc(sqrt_sem, 1)
        if n_sqrts == 2:
            act.wait_ge(red_sem, n)
            act.activation(gn[:, split_g:], gs_all[:, split_g:], Act.Sqrt,
                           accum_out=pr2[:, 1:2]).then_inc(sqrt_sem, 1)

    def emit_dve(dve):
        for i in range(n):
            dve.wait_ge(sq_sem, i + 1)
            dve.reduce_sum(
                gs_all[:, goffs[i]:goffs[i] + gpcs[i]],
                t[i % B][:, :schedule[i]].rearrange("p (g k) -> p g k",
                                                    k=GROUP),
                axis=mybir.AxisListType.X,
            ).then_inc(red_sem, 1)
        dve.wait_ge(mm_sem, 1)
        dve.tensor_copy(res_sb, ps).then_inc(cp_sem, 1)

    def emit_pe(pe):
        pe.wait_ge(ones_sem, 1)
        pe.wait_ge(sqrt_sem, n_sqrts)
        pe.matmul(ps, ones, pr2, start=True, stop=True).then_inc(mm_sem, 1)

    if use_block:
        with nc.Block(no_gpsimd_drain=True) as block:
            block.gpsimd(emit_gp)
            block.sync(emit_sp)
            block.scalar(emit_act)
            block.vector(emit_dve)
            block.tensor(emit_pe)
    else:
        emit_gp(nc.gpsimd)
        emit_sp(nc.sync)
        emit_act(nc.scalar)
        emit_dve(nc.vector)
        emit_pe(nc.tensor)

    nc.compile()
    return nc


def kernel(weight, c_omega):
    global _compiled, LAST_RESULTS
    from concourse.bass_utils import run_bass_kernel_spmd

    if _compiled is None:
        _compiled = build()
    nc = _compiled

    w = np.asarray(weight)
    if w.dtype != np.float32:
        w = w.astype(np.float32)
    w = np.ascontiguousarray(w)
    flat = w.reshape(-1)
    per_core = flat.size // N_CORES
    in_maps = [
        {"x": flat[c * per_core:(c + 1) * per_core].reshape(P, F_PER_PART)}
        for c in range(N_CORES)
    ]
    LAST_RESULTS = run_bass_kernel_spmd(nc, in_maps,
                                        core_ids=list(range(N_CORES)))
    total = 0.0
    for r in LAST_RESULTS.results:
        total += float(r["out"].astype(np.float64).sum())
    loss = total / N_ROWS * (C_OMEGA * float(c_omega))
    return np.float32(loss)


def selftest_sim(f_per_part=625, schedule=(250, 250, 75, 25, 25),
                 in_bufs=3, seed=0, **kw):
    """CoreSim check on a scaled-down instance; returns max rel err."""
    from concourse.bass_interp import CoreSim

    nc = build(f_per_part=f_per_part, schedule=list(schedule),
               in_bufs=in_bufs, **kw)
    rng = np.random.default_rng(seed)
    xv = rng.standard_normal((P, f_per_part)).astype(np.float32)
    sim = CoreSim(nc)
    sim.tensor("x")[:] = xv
    sim.simulate()
    got = float(np.array(sim.tensor("out")).astype(np.float64).sum())
    g = xv.reshape(P, f_per_part // GROUP, GROUP)
    want = float(np.sqrt((g.astype(np.float64) ** 2).sum(-1)).sum())
    return abs(got - want) / abs(want)
